# revision 7
# baseline (speedup 1.0000x reference)
"""NT-Xent loss kernel for 8 TRN2 NeuronCores (Bass/Tile).

Computes: reps = l2norm(concat(z_i, z_j)); sim = reps @ reps.T / T;
e = exp(sim); lse_i = logsumexp over off-diagonal e-row; pos_i = e[i, i+-B];
loss = mean(lse - pos).

Strategy (data-parallel rows, fully fused on-chip — sim is never
materialized in DRAM):
  - Host: l2-normalize, transpose to [D=128, 2B=16384].
  - Each core c gets a column-ROTATED copy (roll by -c*2048) so its own
    2048 row-vectors sit in rotated chunk 0.  This makes the diagonal
    (self-similarity) block land at compile-time-known columns for every
    core: one SPMD program, no runtime branching.
  - Per 128-row block: 32 matmuls [128,512] -> PSUM, ACT exp(sim/T) ->
    e tiles in SBUF, DVE row-max, ACT second exp(e - max) with
    per-partition bias and accum_out row-sums, lse = max + ln(sum).
  - Positives are e[p, 8192 + diag] — extracted from the already-computed
    e tiles with an identity-mask multiply + row-sum reduce.
  - Host: loss = (sum(lse) - sum(pos)) / 16384.
"""

import os
import numpy as np

TEMP = 0.07
B = 8192
D = 128
N = 2 * B            # 16384 rows/cols of sim
NCORES = 8
ROWS_PER_CORE = N // NCORES   # 2048
BLKS = ROWS_PER_CORE // 128   # 16 row-blocks per core
CHUNK = 2048                  # SBUF column chunk
NCHUNK = N // CHUNK           # 8
OUT_LEN = ROWS_PER_CORE + 128  # lse rows + per-partition pos accumulator

_cache = {}


def build_nc():
    """Build the SPMD Bass program (identical for all cores)."""
    import concourse.bacc as bacc
    import concourse.bass as bass
    import concourse.mybir as mybir
    import concourse.tile as tile

    f32 = mybir.dt.float32
    AF = mybir.ActivationFunctionType
    ALU = mybir.AluOpType

    nc = bacc.Bacc(
        "TRN2",
        target_bir_lowering=False,
        debug=False,
        num_devices=NCORES,
    )

    zt_d = nc.dram_tensor("zt", [D, N], mybir.dt.float32r, kind="ExternalInput").ap()
    dmask_d = nc.dram_tensor("dmask", [128, 128], f32, kind="ExternalInput").ap()
    eye_d = nc.dram_tensor("eye", [128, 128], f32, kind="ExternalInput").ap()
    out_d = nc.dram_tensor("out", [OUT_LEN], f32, kind="ExternalOutput").ap()

    bf16 = mybir.dt.bfloat16

    with tile.TileContext(nc) as tc:
        with (
            tc.tile_pool(name="rpool", bufs=NCHUNK) as rpool,
            tc.tile_pool(name="cpool", bufs=1) as cpool,
            tc.tile_pool(name="epool", bufs=2 * NCHUNK + 2) as epool,
            tc.tile_pool(name="scrpool", bufs=3) as scrpool,
            tc.tile_pool(name="spool", bufs=6) as spool,
            tc.tile_pool(name="psum", bufs=2, space=bass.MemorySpace.PSUM) as psumpool,
        ):
            # ---- load persistent data ----
            R = []
            for q in range(NCHUNK):
                rq = rpool.tile([D, CHUNK], mybir.dt.float32r, tag="rchunk")
                nc.sync.dma_start(rq[:], zt_d[:, q * CHUNK:(q + 1) * CHUNK])
                R.append(rq)
            dmask = cpool.tile([128, 128], f32, tag="dmask")
            nc.sync.dma_start(dmask[:], dmask_d[:])
            eye = cpool.tile([128, 128], f32, tag="eye")
            nc.sync.dma_start(eye[:], eye_d[:])

            posacc = cpool.tile([128, 1], f32, tag="posacc")
            nc.vector.memset(posacc[:], 0.0)
            mstage = cpool.tile([128, BLKS], f32, tag="mstage")
            sstage = cpool.tile([128, BLKS], f32, tag="sstage")
            lsestage = cpool.tile([128, BLKS], f32, tag="lsestage")

            # ---- main loop: 16 row-blocks, exp2 software-pipelined one
            # block behind exp1 so ACT never waits on the row-max ----
            prev = None  # (etiles, nm, lm) of the previous block

            def emit_exp2(state):
                petiles, pnm, plm = state
                sacc = spool.tile([128, NCHUNK], f32, tag="sacc")
                for q in range(NCHUNK):
                    scr = scrpool.tile([128, CHUNK], bf16, tag="scr")
                    nc.scalar.activation(
                        scr[:], petiles[q][:], AF.Exp, bias=pnm[:], scale=1.0
                    )
                    nc.vector.reduce_sum(
                        sacc[:, q:q + 1], scr[:], axis=mybir.AxisListType.X
                    )
                nc.vector.reduce_sum(
                    sstage[:, plm:plm + 1], sacc[:], axis=mybir.AxisListType.X
                )

            for lm in range(BLKS):
                lhsT = R[0][:, lm * 128:(lm + 1) * 128]  # this core's rows
                emax = spool.tile([128, NCHUNK], f32, tag="emax")
                etiles = []
                for q in range(NCHUNK):
                    ps = psumpool.tile([128, CHUNK], f32, tag="ps")
                    for t in range(4):
                        nc.tensor.matmul(
                            ps[:, t * 512:(t + 1) * 512],
                            lhsT,
                            R[q][:, t * 512:(t + 1) * 512],
                            start=True,
                            stop=True,
                        )
                    eq = epool.tile([128, CHUNK], bf16, tag="echunk")
                    # e = exp(sim / T)
                    nc.scalar.activation(eq[:], ps[:], AF.Exp, scale=1.0 / TEMP)
                    if q == 0:
                        # zero out own diagonal (self-similarity)
                        nc.vector.tensor_tensor(
                            eq[:, lm * 128:(lm + 1) * 128],
                            eq[:, lm * 128:(lm + 1) * 128],
                            dmask[:],
                            op=ALU.mult,
                        )
                    if q == 4:
                        # positives live at cols 8192 + (lm*128 + p)
                        pw = spool.tile([128, 128], f32, tag="pw")
                        nc.vector.tensor_tensor(
                            pw[:],
                            eq[:, lm * 128:(lm + 1) * 128],
                            eye[:],
                            op=ALU.mult,
                        )
                        pr = spool.tile([128, 1], f32, tag="pr")
                        nc.vector.reduce_sum(pr[:], pw[:], axis=mybir.AxisListType.X)
                        nc.vector.tensor_add(posacc[:], posacc[:], pr[:])
                    nc.vector.reduce_max(
                        emax[:, q:q + 1], eq[:], axis=mybir.AxisListType.X
                    )
                    etiles.append(eq)

                m = mstage[:, lm:lm + 1]
                nc.vector.reduce_max(m, emax[:], axis=mybir.AxisListType.X)
                nm = spool.tile([128, 1], f32, tag="nm")
                nc.vector.tensor_scalar_mul(nm[:], m, -1.0)

                if prev is not None:
                    emit_exp2(prev)
                prev = (etiles, nm, lm)

            emit_exp2(prev)

            # lse = m + ln(s), batched over all blocks (single Ln — avoids
            # per-block exp<->ln ACT table switching)
            nc.scalar.activation(lsestage[:], sstage[:], AF.Ln)
            nc.vector.tensor_add(lsestage[:], lsestage[:], mstage[:])

            # ---- outputs ----
            # out[f*128 + p] = lsestage[p, f]
            nc.sync.dma_start(
                out_d[0:ROWS_PER_CORE].rearrange("(f p) -> p f", p=128),
                lsestage[:],
            )
            nc.sync.dma_start(
                out_d[ROWS_PER_CORE:OUT_LEN].rearrange("(p o) -> p o", o=1),
                posacc[:],
            )

    nc.compile()
    return nc


def make_in_maps(z_i: np.ndarray, z_j: np.ndarray):
    Z = np.concatenate([np.asarray(z_i), np.asarray(z_j)], axis=0).astype(np.float32)
    nrm = np.linalg.norm(Z, axis=1, keepdims=True)
    R = (Z / np.maximum(nrm, 1e-12)).astype(np.float32)
    RT = np.ascontiguousarray(R.T)  # [128, 16384]
    eye = np.eye(128, dtype=np.float32)
    dmask = (1.0 - eye).astype(np.float32)
    # FP32r (tf32-style) mantissa rounding: PE consumes 10-bit mantissa.
    # Round-to-nearest (add half-ULP, carry propagates into the exponent),
    # NOT truncation — truncation systematically shrinks every similarity.
    bits = RT.view(np.uint32)
    bits += np.uint32(0x1000)
    bits &= np.uint32(0xFFFFE000)
    in_maps = []
    for c in range(NCORES):
        zt = np.ascontiguousarray(np.roll(RT, -c * ROWS_PER_CORE, axis=1))
        in_maps.append({"zt": zt, "dmask": dmask, "eye": eye})
    return in_maps


def kernel(z_i: np.ndarray, z_j: np.ndarray) -> np.ndarray:
    from concourse.bass_utils import run_bass_kernel_spmd

    if "nc" not in _cache:
        _cache["nc"] = build_nc()
    nc = _cache["nc"]

    in_maps = make_in_maps(z_i, z_j)
    res = run_bass_kernel_spmd(
        nc,
        in_maps,
        core_ids=list(range(NCORES)),
        trace=bool(int(os.environ.get("NTX_TRACE", "0"))),
    )
    _cache["last_result"] = res

    lse_sum = 0.0
    pos_sum = 0.0
    for c in range(NCORES):
        out = res.results[c]["out"].astype(np.float64)
        lse_sum += out[:ROWS_PER_CORE].sum()
        pos_sum += out[ROWS_PER_CORE:].sum()
    loss = (lse_sum - pos_sum) / float(N)
    return np.float32(loss)


# revision 12
# speedup vs baseline: 1.2118x; 1.2118x over previous
"""NT-Xent loss kernel for 8 TRN2 NeuronCores (Bass/Tile).

Computes: reps = l2norm(concat(z_i, z_j)); sim = reps @ reps.T / T;
e = exp(sim); lse_i = logsumexp over off-diagonal e-row; pos_i = e[i, i+-B];
loss = mean(lse - pos).

Strategy (data-parallel rows, fully fused on-chip — sim is never
materialized in DRAM):
  - Host: l2-normalize, transpose to [D=128, 2B=16384].
  - Each core c gets a column-ROTATED copy (roll by -c*2048) so its own
    2048 row-vectors sit in rotated chunk 0.  This makes the diagonal
    (self-similarity) block land at compile-time-known columns for every
    core: one SPMD program, no runtime branching.
  - Per 128-row block: 32 matmuls [128,512] -> PSUM, ACT exp(sim/T) ->
    e tiles in SBUF, DVE row-max, ACT second exp(e - max) with
    per-partition bias and accum_out row-sums, lse = max + ln(sum).
  - Positives are e[p, 8192 + diag] — extracted from the already-computed
    e tiles with an identity-mask multiply + row-sum reduce.
  - Host: loss = (sum(lse) - sum(pos)) / 16384.
"""

import os
import numpy as np

TEMP = 0.07
B = 8192
D = 128
N = 2 * B            # 16384 rows/cols of sim
NCORES = 8
ROWS_PER_CORE = N // NCORES   # 2048
BLKS = ROWS_PER_CORE // 128   # 16 row-blocks per core
CHUNK = 2048                  # SBUF column chunk
NCHUNK = N // CHUNK           # 8
OUT_LEN = ROWS_PER_CORE + 128  # lse rows + per-partition pos accumulator

_cache = {}


def build_nc():
    """Build the SPMD Bass program (identical for all cores)."""
    import concourse.bacc as bacc
    import concourse.bass as bass
    import concourse.mybir as mybir
    import concourse.tile as tile

    f32 = mybir.dt.float32
    AF = mybir.ActivationFunctionType
    ALU = mybir.AluOpType

    nc = bacc.Bacc(
        "TRN2",
        target_bir_lowering=False,
        debug=False,
        num_devices=NCORES,
    )

    zt_d = nc.dram_tensor("zt", [D, N], mybir.dt.float32r, kind="ExternalInput").ap()
    dmask_d = nc.dram_tensor("dmask", [128, 128], f32, kind="ExternalInput").ap()
    eye_d = nc.dram_tensor("eye", [128, 128], f32, kind="ExternalInput").ap()
    out_d = nc.dram_tensor("out", [OUT_LEN], f32, kind="ExternalOutput").ap()

    bf16 = mybir.dt.bfloat16

    with tile.TileContext(nc) as tc:
        with (
            tc.tile_pool(name="rpool", bufs=NCHUNK) as rpool,
            tc.tile_pool(name="cpool", bufs=1) as cpool,
            tc.tile_pool(name="epool", bufs=1) as epool,
            tc.tile_pool(name="spool", bufs=6) as spool,
            tc.tile_pool(name="psum", bufs=2, space=bass.MemorySpace.PSUM) as psumpool,
        ):
            # ---- load persistent data ----
            R = []
            for q in range(NCHUNK):
                rq = rpool.tile([D, CHUNK], mybir.dt.float32r, tag="rchunk")
                nc.sync.dma_start(rq[:], zt_d[:, q * CHUNK:(q + 1) * CHUNK])
                R.append(rq)
            dmask = cpool.tile([128, 128], f32, tag="dmask")
            nc.sync.dma_start(dmask[:], dmask_d[:])
            eye = cpool.tile([128, 128], f32, tag="eye")
            nc.sync.dma_start(eye[:], eye_d[:])

            posacc = cpool.tile([128, 1], f32, tag="posacc")
            nc.vector.memset(posacc[:], 0.0)
            mstage = cpool.tile([128, BLKS], f32, tag="mstage")
            sstage = cpool.tile([128, BLKS], f32, tag="sstage")
            lsestage = cpool.tile([128, BLKS], f32, tag="lsestage")

            # Three rotating full-width bf16 e buffers: exp1(b) fills
            # ebuf[b%3]; exp2(b) reads it and writes ebuf[(b+2)%3] (free at
            # that point), so exp2 of block b overlaps exp1 of block b+1.
            ebufs = [
                epool.tile([128, N], bf16, tag=f"ebuf{i}", name=f"ebuf{i}")
                for i in range(3)
            ]

            # ---- main loop: 16 row-blocks, exp2 software-pipelined one
            # block behind exp1 so ACT never waits on the row-max ----
            prev = None  # (e, nm, lm) of the previous block

            def emit_exp2(state):
                pe, pnm, plm = state
                nc.scalar.activation(
                    ebufs[(plm + 2) % 3][:],
                    pe[:],
                    AF.Exp,
                    bias=pnm[:],
                    scale=1.0,
                    accum_out=sstage[:, plm:plm + 1],
                )

            for lm in range(BLKS):
                lhsT = R[0][:, lm * 128:(lm + 1) * 128]  # this core's rows
                e = ebufs[lm % 3]
                emax = spool.tile([128, NCHUNK], f32, tag="emax")
                for q in range(NCHUNK):
                    ps = psumpool.tile([128, CHUNK], f32, tag="ps")
                    for t in range(4):
                        nc.tensor.matmul(
                            ps[:, t * 512:(t + 1) * 512],
                            lhsT,
                            R[q][:, t * 512:(t + 1) * 512],
                            start=True,
                            stop=True,
                        )
                    eq = e[:, q * CHUNK:(q + 1) * CHUNK]
                    # e = exp(sim / T)
                    nc.scalar.activation(eq, ps[:], AF.Exp, scale=1.0 / TEMP)
                    if q == 0:
                        # zero out own diagonal (self-similarity)
                        nc.vector.tensor_tensor(
                            e[:, lm * 128:(lm + 1) * 128],
                            e[:, lm * 128:(lm + 1) * 128],
                            dmask[:],
                            op=ALU.mult,
                        )
                    if q == 4:
                        # positives live at cols 8192 + (lm*128 + p)
                        pw = spool.tile([128, 128], f32, tag="pw")
                        nc.vector.tensor_tensor(
                            pw[:],
                            e[:, 8192 + lm * 128:8192 + (lm + 1) * 128],
                            eye[:],
                            op=ALU.mult,
                        )
                        pr = spool.tile([128, 1], f32, tag="pr")
                        nc.vector.reduce_sum(pr[:], pw[:], axis=mybir.AxisListType.X)
                        nc.vector.tensor_add(posacc[:], posacc[:], pr[:])
                    nc.vector.reduce_max(
                        emax[:, q:q + 1], eq, axis=mybir.AxisListType.X
                    )

                m = mstage[:, lm:lm + 1]
                nc.vector.reduce_max(m, emax[:], axis=mybir.AxisListType.X)
                nm = spool.tile([128, 1], f32, tag="nm")
                nc.vector.tensor_scalar_mul(nm[:], m, -1.0)

                if prev is not None:
                    emit_exp2(prev)
                prev = (e, nm, lm)

            emit_exp2(prev)

            # lse = m + ln(s), batched over all blocks (single Ln — avoids
            # per-block exp<->ln ACT table switching)
            nc.scalar.activation(lsestage[:], sstage[:], AF.Ln)
            nc.vector.tensor_add(lsestage[:], lsestage[:], mstage[:])

            # ---- outputs ----
            # out[f*128 + p] = lsestage[p, f]
            nc.sync.dma_start(
                out_d[0:ROWS_PER_CORE].rearrange("(f p) -> p f", p=128),
                lsestage[:],
            )
            nc.sync.dma_start(
                out_d[ROWS_PER_CORE:OUT_LEN].rearrange("(p o) -> p o", o=1),
                posacc[:],
            )

    nc.compile()
    return nc


def make_in_maps(z_i: np.ndarray, z_j: np.ndarray):
    Z = np.concatenate([np.asarray(z_i), np.asarray(z_j)], axis=0).astype(np.float32)
    nrm = np.linalg.norm(Z, axis=1, keepdims=True)
    R = (Z / np.maximum(nrm, 1e-12)).astype(np.float32)
    RT = np.ascontiguousarray(R.T)  # [128, 16384]
    eye = np.eye(128, dtype=np.float32)
    dmask = (1.0 - eye).astype(np.float32)
    # FP32r (tf32-style) mantissa rounding: PE consumes 10-bit mantissa.
    # Round-to-nearest (add half-ULP, carry propagates into the exponent),
    # NOT truncation — truncation systematically shrinks every similarity.
    bits = RT.view(np.uint32)
    bits += np.uint32(0x1000)
    bits &= np.uint32(0xFFFFE000)
    in_maps = []
    for c in range(NCORES):
        zt = np.ascontiguousarray(np.roll(RT, -c * ROWS_PER_CORE, axis=1))
        in_maps.append({"zt": zt, "dmask": dmask, "eye": eye})
    return in_maps


def kernel(z_i: np.ndarray, z_j: np.ndarray) -> np.ndarray:
    from concourse.bass_utils import run_bass_kernel_spmd

    if "nc" not in _cache:
        _cache["nc"] = build_nc()
    nc = _cache["nc"]

    in_maps = make_in_maps(z_i, z_j)
    res = run_bass_kernel_spmd(
        nc,
        in_maps,
        core_ids=list(range(NCORES)),
        trace=bool(int(os.environ.get("NTX_TRACE", "0"))),
    )
    _cache["last_result"] = res

    lse_sum = 0.0
    pos_sum = 0.0
    for c in range(NCORES):
        out = res.results[c]["out"].astype(np.float64)
        lse_sum += out[:ROWS_PER_CORE].sum()
        pos_sum += out[ROWS_PER_CORE:].sum()
    loss = (lse_sum - pos_sum) / float(N)
    return np.float32(loss)
